# revision 77
# baseline (speedup 1.0000x reference)
"""Two-layer GAT on 8 Trainium2 NeuronCores (Bass/Tile) — v4 (457us).

v3 (485us) -> v4: aligned 64-dst segments. Dst tiles are fixed 128-node
slabs (TT=49, no greedy packing); each tile splits into two aligned 64-dst
segments, and every (tile, segment, src-half) group gets ceil(edges/128)
chunks (count maxed over the 8 cores so one SPMD program serves all).
One-hot matrices shrink 2x: sel is [128e x 64d] and selT is [64d x 128e],
with selT blocks pairing an s=0 chunk (partitions 0:64) and an s=1 chunk
(partitions 64:128) so lhsT partitions line up with the partition-sliced
a_dst rhs. The acc matmul writes the 64-partition PSUM slice of its
segment (skip_group_check for the interleaved per-segment groups).
The acc matmul writes the 64-partition PSUM slice of its
segment; each accumulator is bracketed by full-partition zero matmuls
(opener before any slice, closer after the last) so every cross-engine
dependency anchors on a trivially-tracked full-tile write — the sliced
writes in between are ordered by the in-order PE stream. Gather fill
drops 92%->85% (+15us) but the sel|selT stream halves (164us -> 88us):
DMA-engine busy 442 -> 378us.

Carried from v3: per-edge logits in PSUM (selT x a_dst matmul + identity x
a_src accumulate, ACT Prelu straight from PSUM), pair-width exp on ACT
broadcast inside the 2x-mode DVE multiply, fp8 t2 rows (64 e4m3 values +
asrc2 as raw f16 bytes, 7ns/desc gather floor), one shared sel|selT host
tensor for both layers, alternating SP/ACT store queues, f16 out rows.
Rel err 1.34e-2 (fp8 t2 dominates; gate 2e-2, inputs deterministic).
"""

import os

import numpy as np

import concourse.bass as bass
import concourse.bacc as bacc
import concourse.tile as tile
import concourse.mybir as mybir
from concourse.bass_utils import run_bass_kernel_spmd
from concourse.masks import make_identity

F32 = mybir.dt.float32
F16 = mybir.dt.float16
F8 = mybir.dt.float8e4
I16 = mybir.dt.int16
I32 = mybir.dt.int32
A = mybir.AluOpType
AF = mybir.ActivationFunctionType
NP_F8 = mybir.dt.np(F8)

# -------- problem constants --------
N, E, IN, HID, OUT, H = 50000, 800000, 128, 32, 64, 8
C1 = H * HID  # 256
NCORES = 8
NPC = N // NCORES        # 6250 dst nodes per core
RPC = 6272               # T1 rows per core (6250 padded to 49*128)
NR = NCORES * RPC        # 50176 T1 rows
T1_LO = 4 * RPC          # 25088: rows of cores 0-3
T1_W = 384               # T1 row stride (768B) — gather granularity
CHL = 9                  # chunks per (tile, half)
CH = 2 * CHL             # chunks per gather call
CALLW = CH * 128         # 2304 edge slots per call
EPS = 1e-16
NEG = 0.2


def _row_of(n):
    """T1 row of node n (cores own contiguous 6250-node ranges, padded)."""
    c = n // NPC
    return c * RPC + (n - c * NPC)


# ---------------------------------------------------------------------------
# host-side preprocessing
# ---------------------------------------------------------------------------

def _prep_weights(W1, as1, ad1, b1, W2, as2, ad2, b2):
    As = np.zeros((C1, H), np.float32)
    Ad = np.zeros((C1, H), np.float32)
    for h in range(H):
        As[h * HID:(h + 1) * HID, h] = as1[h]
        Ad[h * HID:(h + 1) * HID, h] = ad1[h]
    W1ext = np.concatenate([W1, W1 @ As, W1 @ Ad], axis=1)  # [128, 272]
    iotarep = np.zeros((128, 128 * CH), np.float16)
    for d in range(128):
        iotarep[:, d * CH:(d + 1) * CH] = d
    b1ext = np.zeros((272,), np.float32)
    b1ext[:C1] = b1
    vs = W2 @ as2[0]   # [256]
    vd = W2 @ ad2[0]   # [256]
    W2ext = np.concatenate([W2, vs[:, None], vd[:, None]], axis=1)  # [256,66]
    b2eff = np.concatenate([b2 - W2.sum(0), [-vs.sum()], [-vd.sum()]])  # [66]
    return {
        "W1ext": W1ext.astype(np.float16),
        "W2ext": W2ext.astype(np.float16),
        "b1ext": np.tile(b1ext[None, :], (128, 1)).astype(np.float16),
        "b2e66": np.tile(b2eff[None, :], (128, 1)).astype(np.float16),
        "iotarep": iotarep,
    }


SEG = 64                      # dst-locals per segment (2 per 128-dst tile)
TT = (NPC + 127) // 128       # 49 fixed, aligned dst tiles per core
PRS = (TT + 1) // 2           # tile pairs (last pair may be a single tile)


def _core_edges(c, src, dst):
    base = c * NPC
    own = (dst >= base) & (dst < base + NPC)
    s = src[own].astype(np.int64)
    d = (dst[own] - base).astype(np.int64)
    order = np.argsort(d, kind="stable")
    return s[order], d[order]


def _group_positions(s, d):
    """Edge positions (into the dst-sorted arrays) per (tile, segment, half).
    half 0 = src on cores 0-3, which is the lo half of BOTH layers' tables."""
    ptr = np.zeros(NPC + 1, np.int64)
    np.cumsum(np.bincount(d, minlength=NPC), out=ptr[1:])
    lom = s < (N // 2)
    pos = {}
    for t in range(TT):
        for sg in (0, 1):
            lo = t * 128 + sg * SEG
            hi = min(lo + SEG, NPC)
            if lo >= NPC:
                continue
            e0, e1 = ptr[lo], ptr[hi]
            idx = np.arange(e0, e1)
            m = lom[e0:e1]
            pos[(t, sg, 0)] = idx[m]
            pos[(t, sg, 1)] = idx[~m]
    return pos


def _make_plan(all_pos):
    """Shared (all-core) call structure: chunk counts are max over cores so
    one SPMD program serves every core."""
    K = {}
    for key in all_pos[0]:
        K[key] = max((len(p[key]) + 127) // 128 for p in all_pos)
    calls, cj, nblk = [], [], []
    occ = {}
    for pr in range(PRS):
        tiles = [t for t in (2 * pr, 2 * pr + 1) if t < TT]
        for hf in (0, 1):
            chunks = []
            scnt = [0, 0]
            for k, t in enumerate(tiles):
                for sg in (0, 1):
                    for _ in range(K.get((t, sg, hf), 0)):
                        chunks.append([k, t, sg, scnt[sg], False, False])
                        occ.setdefault((t, sg), []).append(
                            (len(calls), len(chunks) - 1))
                        scnt[sg] += 1
            calls.append(chunks)
            cj.append(len(chunks))
            nblk.append(max(scnt) if chunks else 0)
    for lst in occ.values():
        g0, j0 = lst[0]
        g1, j1 = lst[-1]
        calls[g0][j0][4] = True   # start of the (t, sg) PSUM group
        calls[g1][j1][5] = True   # stop
    idx_off = np.concatenate(([0], np.cumsum([c * 8 for c in cj])))
    scp_cols = [cj[g] * SEG + nblk[g] * 128 for g in range(len(cj))]
    scp_off = np.concatenate(([0], np.cumsum(scp_cols)))
    return {
        "TT": TT, "PRS": PRS, "K": K, "calls": calls, "cj": cj,
        "nblk": nblk, "idx_off": idx_off.astype(int),
        "scp_cols": scp_cols, "scp_off": scp_off.astype(int),
        "cjmax": max(cj), "scmax": max(scp_cols),
        "idx_tot": int(idx_off[-1]), "scp_tot": int(scp_off[-1]),
    }


def _pack_core(plan, pos, srow1, srow2, d):
    """Both layers' gather idx streams + the shared sel|selT fp8 stream.
    Per call: [sel: cj*64 cols (e-part, chunk-major)] then [selT blocks:
    nblk*128 cols; block b holds an s=0 chunk on partitions 0:64 and an
    s=1 chunk on partitions 64:128, so lhsT partitions match the adt rhs]."""
    K = plan["K"]
    t2lo = 4 * TT * 128
    idx1 = np.zeros((128, plan["idx_tot"]), np.int16)
    idx2 = np.zeros((128, plan["idx_tot"]), np.int16)
    scp = np.zeros((128, plan["scp_tot"]), NP_F8)
    iot = np.arange(SEG)
    for g, chunks in enumerate(plan["calls"]):
        cjg = plan["cj"][g]
        if cjg == 0:
            continue
        hf = g % 2
        rows1 = np.zeros((cjg * 128,), np.int64)
        rows2 = np.zeros((cjg * 128,), np.int64)
        sdl = np.full((cjg * 128,), -1, np.int64)
        jj = 0
        seen = set()
        for k, t, sg, b, st, sp in chunks:
            if (t, sg) in seen:
                continue
            seen.add((t, sg))
            p = pos.get((t, sg, hf), np.zeros(0, np.int64))
            o = jj * 128
            rows1[o:o + len(p)] = srow1[p] - (T1_LO if hf else 0)
            rows2[o:o + len(p)] = srow2[p] - (t2lo if hf else 0)
            sdl[o:o + len(p)] = d[p] - (t * 128 + sg * SEG)
            jj += K[(t, sg, hf)]
        o0, o1 = plan["idx_off"][g], plan["idx_off"][g + 1]
        idx1[:, o0:o1] = np.tile(
            rows1.reshape(cjg * 8, 16).T.astype(np.int16), (8, 1))
        idx2[:, o0:o1] = np.tile(
            rows2.reshape(cjg * 8, 16).T.astype(np.int16), (8, 1))
        so = plan["scp_off"][g]
        sdl2 = sdl.reshape(cjg, 128)
        sel = (sdl2[:, :, None] == iot[None, None, :])      # [jj, e, dseg]
        scp[:, so:so + cjg * SEG] = sel.transpose(1, 0, 2).reshape(
            128, cjg * SEG).astype(NP_F8)
        sto = so + cjg * SEG
        for jjx, (k, t, sg, b, st, sp) in enumerate(chunks):
            oh = (sdl2[jjx][None, :] == iot[:, None])        # [dseg, e]
            scp[sg * SEG:(sg + 1) * SEG,
                sto + b * 128:sto + (b + 1) * 128] = oh.astype(NP_F8)
    return idx1, idx2, scp


def host_prep(inputs):
    wd = _prep_weights(
        np.asarray(inputs["W1"], np.float32),
        np.asarray(inputs["att_src1"], np.float32),
        np.asarray(inputs["att_dst1"], np.float32),
        np.asarray(inputs["b1"], np.float32),
        np.asarray(inputs["W2"], np.float32),
        np.asarray(inputs["att_src2"], np.float32),
        np.asarray(inputs["att_dst2"], np.float32),
        np.asarray(inputs["b2"], np.float32),
    )
    ei = np.asarray(inputs["edge_index"]).astype(np.int64)
    loops = np.arange(N, dtype=np.int64)
    src = np.concatenate([ei[0], loops])
    dst = np.concatenate([ei[1], loops])
    x = np.asarray(inputs["x"], np.float32).astype(np.float16)

    cores = [_core_edges(c, src, dst) for c in range(NCORES)]
    poss = [_group_positions(s, d) for s, d in cores]
    plan = _make_plan(poss)

    # aligned tiles: slot of node (c, local n) is simply c*TT*128 + n
    slotrow = np.zeros(N, np.int64)
    for c in range(NCORES):
        slotrow[c * NPC:(c + 1) * NPC] = c * TT * 128 + np.arange(NPC)

    # adidx: per-slot a_dst gather rows (clamped to real nodes)
    rows = np.minimum(np.arange(TT * 128), NPC - 1)
    adidx = np.tile(rows.reshape(TT * 8, 16).T.astype(np.int16), (8, 1))

    per_core = []
    for c in range(NCORES):
        s, d = cores[c]
        i1, i2, scp = _pack_core(plan, poss[c], _row_of(s), slotrow[s], d)
        xc = np.zeros((IN, RPC), np.float16)
        xc[:, :NPC] = x[c * NPC:(c + 1) * NPC].T
        per_core.append({"g1idx": i1, "g2idx": i2, "selT1": scp,
                         "adidx": adidx, "xTc": xc})
    common = dict(wd)
    common["slotrow"] = slotrow
    return plan, common, per_core


# ---------------------------------------------------------------------------
# device program
# ---------------------------------------------------------------------------

def _gather_raw(eng, out_ap, in_ap, idxs_ap, num_idxs, elem_size, elem_step):
    """dma_gather with elem_size_bytes not a multiple of 256B (non-transpose
    path only; the 256B rule is a transpose-mode restriction — the Q7 kernel
    packets arbitrary elem sizes, only the row stride is encoded in 256B
    units).  Mirrors BassGpSimd.dma_gather's construction."""
    from concourse.ap_utils import ap_is_contiguous
    import concourse.mybir as mb
    assert idxs_ap.dtype == mybir.dt.int16
    assert in_ap.dtype == out_ap.dtype
    elem_size_bytes = elem_size * mybir.dt.size(in_ap.dtype)
    assert in_ap.ap[-1][1] == out_ap.ap[-1][1] == elem_size
    assert ap_is_contiguous(out_ap.ap[1:])
    assert ap_is_contiguous(idxs_ap.ap[1:])
    assert in_ap.ap[0][0] == elem_step
    stride_bytes = elem_step * mybir.dt.size(in_ap.dtype)
    assert stride_bytes % 256 == 0 and stride_bytes // 256 < 256
    _in_ap = eng.lower_ap_dma(in_ap, for_custom_bir_dma=True)
    _idxs_ap = eng.lower_ap(idxs_ap)
    _out_ap = eng.lower_ap(out_ap)
    return eng.add_instruction(
        mb.InstDMAGatherAnt(
            name=eng.bass.get_next_instruction_name(),
            ins=[*_in_ap, _idxs_ap,
                 eng.lower_val_access(eng.to_reg(num_idxs))],
            outs=[_out_ap],
            transpose=False,
            num_idxs=num_idxs,
            elem_size=elem_size,
            stride_bytes_256=stride_bytes // 256,
            gen_mode=0,
            single_packet=False,
            queue_num=0,
            sbuf_tokens_per_rank=0,
            sbuf_free_dim_per_rank=0,
            sbuf_free_dim_pad_per_rank=0,
            sbuf_byte_offset=0,
        )
    )


def build_nc(plan, num_devices=NCORES, with_collective=True, phases="ABCD",
             dbg=False):
    nc = bacc.Bacc("TRN2", target_bir_lowering=False, debug=False,
                   num_devices=num_devices)
    dt = nc.dram_tensor
    xTc = dt("xTc", [IN, RPC], F16, kind="ExternalInput").ap()
    W1ext = dt("W1ext", [128, 272], F16, kind="ExternalInput").ap()
    W2ext = dt("W2ext", [256, 66], F16, kind="ExternalInput").ap()
    b1ext = dt("b1ext", [128, 272], F16, kind="ExternalInput").ap()
    b2e66 = dt("b2e66", [128, 66], F16, kind="ExternalInput").ap()
    g1idx = dt("g1idx", [128, plan["idx_tot"]], I16, kind="ExternalInput").ap()
    g2idx = dt("g2idx", [128, plan["idx_tot"]], I16, kind="ExternalInput").ap()
    selT1 = dt("selT1", [128, plan["scp_tot"]], F8, kind="ExternalInput").ap()
    adidx = dt("adidx", [128, TT * 8], I16, kind="ExternalInput").ap()
    t1slice = dt("t1slice", [RPC, T1_W], F16, kind="Internal").ap()
    T1 = dt("T1", [NR, T1_W], F16, kind="Internal",
            addr_space="Shared" if with_collective else "Local").ap()
    t2rows = TT * 128
    # t2 rows are fp8: 64 fp8 h2 values + asrc2 as raw f16 in bytes 64:66,
    # padded to a 256B stride (gather stride must be a 256B multiple). The
    # 66B gather elem rides the 7ns/desc floor instead of f16's 11.6ns.
    t2slice = dt("t2slice", [t2rows, 256], F8, kind="Internal").ap()
    t2full = dt("t2full", [NCORES * t2rows, 256], F8, kind="Internal",
                addr_space="Shared" if with_collective else "Local").ap()
    outp = dt("out", [t2rows, 64], F16, kind="ExternalOutput").ap()
    nc._dbg = None

    with tile.TileContext(nc) as tc:
        with tc.tile_pool(name="consts", bufs=1) as cp:
            W1e_sb = cp.tile([128, 272], F16)
            nc.sync.dma_start(out=W1e_sb[:], in_=W1ext[:])
            W2a_sb = cp.tile([128, 66], F16)
            nc.sync.dma_start(out=W2a_sb[:], in_=W2ext[0:128, :])
            W2b_sb = cp.tile([128, 66], F16)
            nc.sync.dma_start(out=W2b_sb[:], in_=W2ext[128:256, :])
            b1_sb = cp.tile([128, 272], F16)
            nc.sync.dma_start(out=b1_sb[:], in_=b1ext[:])
            b2_sb = cp.tile([128, 66], F16)
            nc.sync.dma_start(out=b2_sb[:], in_=b2e66[:])
            oneall = cp.tile([128, 128], F16)
            nc.vector.memset(oneall[:], 1.0 / 128.0)
            idn = cp.tile([128, 128], F16)
            make_identity(nc, idn[:])
            z128 = cp.tile([128, 128], F16)
            nc.vector.memset(z128[:], 0.0)
            zw = cp.tile([128, 264], F16)
            nc.vector.memset(zw[:], 0.0)
            g1i_sb = cp.tile([128, plan["idx_tot"]], I16)
            nc.sync.dma_start(out=g1i_sb[:], in_=g1idx[:])
            g2i_sb = cp.tile([128, plan["idx_tot"]], I16)
            adidx_sb = cp.tile([128, TT * 8], I16)
            nc.sync.dma_start(out=adidx_sb[:], in_=adidx[:])
            adtall_sb = cp.tile([128, TT, 8], F16)  # bulk a_dst gather target
            adst2_sb = cp.tile([128, TT], F16)  # written in B-fin, read in D

            # ---------------- Phase A: own T1 slice ----------------
            if "A" in phases:
                with tc.tile_pool(name="pa", bufs=2) as pa, \
                     tc.tile_pool(name="paps", bufs=4, space="PSUM") as paps:
                    XB = 2048
                    nblk = (RPC + XB - 1) // XB
                    for blk in range(nblk):
                        n0 = blk * XB
                        bw = min(XB, RPC - n0)
                        nt = bw // 128
                        xb = pa.tile([128, XB], F16, tag="xb", name="xb")
                        nc.sync.dma_start(out=xb[:, 0:bw],
                                          in_=xTc[:, n0:n0 + bw])
                        t1b = pa.tile([128, 16, 272], F16, tag="t1b",
                                      name="t1b")
                        for i in range(nt):
                            ps = paps.tile([128, 272], F32, tag="aps",
                                           name="aps")
                            nc.tensor.matmul(ps[:],
                                             lhsT=xb[:, i * 128:(i + 1) * 128],
                                             rhs=W1e_sb[:], start=True,
                                             stop=False)
                            nc.tensor.matmul(ps[:], lhsT=oneall[:],
                                             rhs=b1_sb[:], start=False,
                                             stop=True)
                            if i % 2 == 0:
                                nc.vector.tensor_copy(t1b[:, i, :], ps[:])
                            else:
                                nc.scalar.copy(t1b[:, i, :], ps[:])
                        nc.sync.dma_start(
                            out=t1slice[n0:n0 + bw, 0:272].rearrange(
                                "(i p) c -> p i c", p=128),
                            in_=t1b[:, 0:nt, :])
                        if not with_collective and "B" in phases:
                            nc.sync.dma_start(
                                out=T1[n0:n0 + bw, 0:272],
                                in_=t1slice[n0:n0 + bw, 0:272])

            # ---------------- AllGather T1 ----------------
            if "B" in phases:
                if with_collective:
                    nc.gpsimd.collective_compute(
                        "AllGather", A.bypass,
                        replica_groups=[list(range(NCORES))],
                        ins=[t1slice[:]], outs=[T1[:]],
                    )
                # bulk a_dst gather: one call for all TT tiles' 128 slots
                _gather_raw(nc.gpsimd, adtall_sb[:],
                            t1slice[0:RPC, 264:272], adidx_sb[:],
                            TT * 128, 8, T1_W)

                # -------------- Phase B: layer-1 aggregation --------------
                _agg_layer(nc, tc, plan, layer=1,
                           tbl_lo=T1[0:T1_LO, 0:264],
                           tbl_hi=T1[T1_LO:NR, 0:264],
                           gidx_sb=g1i_sb, selT_in=selT1,
                           idn=idn, z128=z128, zrhs=zw, oneall=oneall,
                           adtall_sb=adtall_sb,
                           W2a_sb=W2a_sb, W2b_sb=W2b_sb, b2_sb=b2_sb,
                           adst2_sb=adst2_sb,
                           t2slice=t2slice, outp=None)

            if "D" in phases:
                nc.sync.dma_start(out=g2i_sb[:], in_=g2idx[:])

            # ---------------- AllGather T2 ----------------
            if "C" in phases:
                if with_collective:
                    nc.gpsimd.collective_compute(
                        "AllGather", A.bypass,
                        replica_groups=[list(range(NCORES))],
                        ins=[t2slice[:]], outs=[t2full[:]],
                    )
                else:
                    nc.sync.dma_start(out=t2full[0:t2rows, :], in_=t2slice[:])

            # ---------------- Phase D: layer-2 aggregation ----------------
            if "D" in phases:
                _agg_layer(nc, tc, plan, layer=2,
                           tbl_lo=t2full[0:4 * t2rows, 0:68],
                           tbl_hi=t2full[4 * t2rows:8 * t2rows, 0:68],
                           gidx_sb=g2i_sb, selT_in=selT1,
                           idn=idn, z128=z128, zrhs=zw, oneall=None,
                           adtall_sb=None,
                           W2a_sb=None, W2b_sb=None, b2_sb=None,
                           adst2_sb=adst2_sb,
                           t2slice=None, outp=outp)

    nc.compile()
    return nc


def _agg_layer(nc, tc, plan, layer, tbl_lo, tbl_hi, gidx_sb, selT_in,
               idn, z128, zrhs, oneall, adtall_sb, W2a_sb, W2b_sb,
               b2_sb, adst2_sb, t2slice, outp):
    L1 = layer == 1
    GW = 264 if L1 else 68   # gathered elements per row (payload)
    GS = T1_W if L1 else 256  # table row stride in elements
    NH = 8 if L1 else 1
    VC = 256 if L1 else 64
    ACC_W = 264 if L1 else 65
    CJM = plan["cjmax"]
    SCM = plan["scmax"]
    name = f"l{layer}"
    PBB = int(os.environ.get("V2_PBB", "4"))
    ACCB = int(os.environ.get("V2_ACCB", "3"))
    with tc.tile_pool(name=f"pb_{name}", bufs=PBB) as pb, \
         tc.tile_pool(name=f"pf_{name}", bufs=2) as pf, \
         tc.tile_pool(name=f"ps_acc_{name}", bufs=ACCB, space="PSUM") as ps_acc, \
         tc.tile_pool(name=f"ps_ad_{name}", bufs=2, space="PSUM") as ps_ad, \
         tc.tile_pool(name=f"ps_fin_{name}", bufs=2, space="PSUM") as ps_fin:
        for pr in range(plan["PRS"]):
            tiles = [t for t in (2 * pr, 2 * pr + 1) if t < TT]
            accs = [ps_acc.tile([128, ACC_W], F32, tag="acc",
                                name=f"acc_{k}") for k in range(len(tiles))]
            # full-partition zero opener: with the zero closer below, every
            # 64-partition segment write is bracketed by full-tile writes on
            # the in-order PE, so cross-engine deps never rely on partition-
            # slice tracking.
            for k in range(len(tiles)):
                nc.tensor.matmul(accs[k][:], lhsT=z128[:],
                                 rhs=zrhs[:, 0:ACC_W], start=True, stop=False,
                                 skip_group_check=True)
            for hf in (0, 1):
                g = 2 * pr + hf
                chunks = plan["calls"][g]
                cjg = plan["cj"][g]
                if cjg == 0:
                    continue
                scols = plan["scp_cols"][g]
                soff = plan["scp_off"][g]
                # shared sel|selT fp8 stream (identical for both layers):
                # sel blocks [e-part, jj*64+dseg], then selT blocks pairing
                # an s=0 chunk (partitions 0:64) with an s=1 chunk (64:128)
                # so lhsT partitions line up with the adt rhs partitions.
                scp = pb.tile([128, SCM], F8, tag="scp", name="scp", bufs=5)
                nc.sync.dma_start(out=scp[:, 0:scols],
                                  in_=selT_in[:, soff:soff + scols])
                gt = pb.tile([128, CJM, GW], F16 if L1 else F8, tag="gt",
                             name="gt", bufs=5)
                io0 = plan["idx_off"][g]
                _gather_raw(
                    nc.gpsimd, gt[:, 0:cjg], tbl_lo if hf == 0 else tbl_hi,
                    gidx_sb[:, io0:io0 + cjg * 8],
                    cjg * 128, GW, GS)
                # per-edge et = a_dst + a_src entirely in PSUM: the fp8 selT
                # one-hot matmul broadcasts a_dst, then an identity matmul
                # accumulates the gathered a_src columns on top.
                adps = ps_ad.tile([128, CJM, NH], F32, tag="adps",
                                  name="adps")
                asrc_ap = (gt[:, :, 256:264] if L1
                           else gt[:, :, 64:66].bitcast(F16))
                sto = cjg * SEG
                for jj, (k, t, sg, b, st, sp) in enumerate(chunks):
                    p0 = sg * SEG
                    lhsT = scp[p0:p0 + SEG, sto + b * 128:sto + (b + 1) * 128]
                    rhs = (adtall_sb[p0:p0 + SEG, t, :] if L1
                           else adst2_sb[p0:p0 + SEG, t:t + 1])
                    nc.tensor.matmul(adps[:, jj, :], lhsT=lhsT, rhs=rhs,
                                     start=True, stop=False)
                    nc.tensor.matmul(adps[:, jj, :], lhsT=idn[:],
                                     rhs=asrc_ap[:, jj, :], start=False,
                                     stop=True)
                lk = pb.tile([128, CJM, NH], F16, tag="lk", name="lk")
                nc.scalar.activation(lk[:, 0:cjg], adps[:, 0:cjg],
                                     AF.Prelu, alpha=NEG)
                # exp at pair width on ACT (cheap), broadcast to the value
                # width inside the DVE multiply via a stride-0 middle dim —
                # the last dim stays packed so the mult keeps 2x DVE mode.
                exf = pb.tile([128, CJM, NH, 2], F16, tag="exf", name="exf")
                nc.scalar.activation(
                    exf[:, 0:cjg],
                    lk[:, 0:cjg, :, None].to_broadcast([128, cjg, NH, 2]),
                    AF.Exp)
                w = pb.tile([128, CJM, ACC_W], F16, tag="w", name="w")
                nc.vector.tensor_copy(w[:, 0:cjg, VC:ACC_W],
                                      exf[:, 0:cjg, :, 0])
                cph = VC // NH // 2  # 16 (L1) / 32 (L2) value pairs per head
                nc.vector.tensor_tensor(
                    out=w[:, 0:cjg, 0:VC].rearrange(
                        "p j (h k two) -> p j h k two", h=NH, two=2),
                    in0=gt[:, 0:cjg, 0:VC].rearrange(
                        "p j (h k two) -> p j h k two", h=NH, two=2),
                    in1=exf[:, 0:cjg, :, None, :].to_broadcast(
                        [128, cjg, NH, cph, 2]),
                    op=A.mult)
                for jj, (k, t, sg, b, st, sp) in enumerate(chunks):
                    p0 = sg * SEG
                    nc.tensor.matmul(accs[k][p0:p0 + SEG, :],
                                     lhsT=scp[:, jj * SEG:(jj + 1) * SEG],
                                     rhs=w[:, jj, :], start=False, stop=False,
                                     skip_group_check=True)
            # close each accumulator with a full-partition zero-add: PE is
            # in-order, so this serializes every 64-partition segment write
            # and hands the finalize a clean full-tile last-writer dep.
            for k, t in enumerate(tiles):
                nc.tensor.matmul(accs[k][:], lhsT=z128[:], rhs=w[:, 0, :],
                                 start=False, stop=True,
                                 skip_group_check=True)
            for k, t in enumerate(tiles):
                if L1:
                    _fin_l1(nc, t, accs[k], pf, ps_fin, idn, oneall, W2a_sb,
                            W2b_sb, b2_sb, adst2_sb, t2slice)
                else:
                    _fin_l2(nc, t, accs[k], pf, outp)


def _fin_l1(nc, t, acc, pf, ps_fin, idn, oneall, W2a_sb, W2b_sb, b2_sb,
            adst2_sb, t2slice):
    # EPS keeps padded dst rows (den=0) finite — their garbage h1 values are
    # never read, but adst2 must stay finite (0*inf = NaN leaks via selT).
    deps = pf.tile([128, 8], F32, tag="deps", name="deps")
    nc.vector.tensor_scalar_add(deps[:], acc[:, 256:264], EPS)
    rec = pf.tile([128, 8], F32, tag="rec", name="rec")
    nc.vector.reciprocal(rec[:], deps[:])
    h1b = pf.tile([128, 256], F16, tag="h1b", name="h1b")
    nc.vector.tensor_tensor(
        out=h1b[:].rearrange("p (h c) -> p h c", h=8),
        in0=acc[:, 0:256].rearrange("p (h c) -> p h c", h=8),
        in1=rec[:, :, None].to_broadcast([128, 8, 32]),
        op=A.mult)
    if t == 0 and getattr(nc, "_dbg", None):
        accs_sb = pf.tile([128, 264], F32, tag="accdbg", name="accdbg")
        nc.vector.tensor_copy(accs_sb[:, 0:256], acc[:, 0:256])
        nc.vector.tensor_copy(accs_sb[:, 256:264], acc[:, 256:264])
        nc.sync.dma_start(out=nc._dbg["h1dbg"][:], in_=accs_sb[:])
    # ho = elu(h1b) + 1 = relu(h1b) + exp(-relu(-h1b)); the -1 is folded
    # into b2eff via W2ext (v1's ACT-based elu decomposition)
    r1 = pf.tile([128, 256], F16, tag="r1", name="r1")
    nc.scalar.activation(r1[:], h1b[:], AF.Relu, scale=-1.0)
    e1 = pf.tile([128, 256], F16, tag="e1", name="e1")
    nc.scalar.activation(e1[:], r1[:], AF.Exp, scale=-1.0)
    rl = pf.tile([128, 256], F16, tag="rl", name="rl")
    nc.scalar.activation(rl[:], h1b[:], AF.Relu)
    ho = pf.tile([128, 256], F16, tag="ho", name="ho")
    nc.gpsimd.tensor_tensor(out=ho[:], in0=rl[:], in1=e1[:], op=A.add)
    h2ps = ps_fin.tile([128, 66], F32, tag="h2ps", name="h2ps")
    for half in (0, 1):
        tp = ps_fin.tile([128, 128], F16, tag="tp", name="tp", bufs=1)
        nc.tensor.transpose(out=tp[:], in_=ho[:, half * 128:(half + 1) * 128],
                            identity=idn[:])
        hoT = pf.tile([128, 128], F16, tag="hoT", name="hoT")
        if half == 0:
            nc.vector.tensor_copy(hoT[:], tp[:])
        else:
            nc.scalar.copy(hoT[:], tp[:])
        nc.tensor.matmul(h2ps[:], lhsT=hoT[:],
                         rhs=(W2a_sb if half == 0 else W2b_sb)[:],
                         start=half == 0, stop=False)
    nc.tensor.matmul(h2ps[:], lhsT=oneall[:], rhs=b2_sb[:], start=False,
                     stop=True)
    t2r = pf.tile([128, 66], F8, tag="t2r", name="t2r")
    nc.scalar.copy(t2r[:, 0:64], h2ps[:, 0:64])
    nc.scalar.copy(t2r[:, 64:66].bitcast(F16), h2ps[:, 64:65])
    nc.vector.tensor_copy(adst2_sb[:, t:t + 1], h2ps[:, 65:66])
    eng = nc.sync if t % 2 == 0 else nc.scalar
    eng.dma_start(out=t2slice[t * 128:(t + 1) * 128, 0:66], in_=t2r[:])


def _fin_l2(nc, t, acc, pf, outp):
    rec = pf.tile([128, 1], F32, tag="rec2", name="rec2")
    nc.vector.reciprocal(rec[:], acc[:, 64:65])
    ot = pf.tile([128, 64], F16, tag="ot", name="ot")
    nc.vector.tensor_scalar_mul(ot[:], acc[:, 0:64], rec[:, 0:1])
    nc.sync.dma_start(out=outp[t * 128:(t + 1) * 128, :], in_=ot[:])


# ---------------------------------------------------------------------------
# entry point
# ---------------------------------------------------------------------------

def make_in_maps(T, common, per_core):
    in_maps = []
    for c in range(NCORES):
        m = {
            "W1ext": common["W1ext"], "W2ext": common["W2ext"],
            "b1ext": common["b1ext"], "b2e66": common["b2e66"],
        }
        pc = per_core[c]
        m.update({k: pc[k] for k in ("xTc", "g1idx", "g2idx", "selT1",
                                     "adidx")})
        in_maps.append(m)
    return in_maps


def kernel(**inputs):
    T, common, per_core = host_prep(inputs)
    nc = build_nc(T)
    in_maps = make_in_maps(T, common, per_core)
    res = run_bass_kernel_spmd(nc, in_maps, core_ids=list(range(NCORES)))
    allrows = np.concatenate([res.results[c]["out"] for c in range(NCORES)],
                             axis=0)
    return allrows[common["slotrow"]].astype(np.float32)

